# revision 27
# baseline (speedup 1.0000x reference)
"""Trainium2 Bass kernel for nn_MLPSimDirectNormConstructor (gnn adjacency builder).

adj = [uni_adj(ss) | uni_adj(st); uni_adj(ts) | triu(uni_adj(tt))] for
  spatial_nodes [4,4096,32], temporal_nodes [4,512,32].

Sharding: 8 cores = (batch b = c//2, half h = c%2).  Each core produces
  - 16 interleaved 128-row blocks of the [ss|st] region (rows 128g, g in GL[h])
  - 256 rows of the [ts|tt] region (rows h*256 .. h*256+256)
The interleaved row-block assignment (g%4 in {2h,2h+1}) makes the
upper-triangle-only abs-max scan of the antisymmetric ss block both
load-balanced and SPMD-uniform.

v3 structure:
  - host does all x transposes (layout prep only)
  - ONE AllReduce(max) of a [128,16] buffer carrying the st/ts/tt maxes
    (cols 0:3) and the 13 per-tile ss abs-maxes (cols 3:16); the ss global
    max is reduced from the collective result.  (v2 used two serialized
    collectives; the second one's trigger+mesh cost ~35us of exposed
    latency.)
  - outputs fp16, uv factors bf16, PSUM as 2x[128,2048], one tanh
    activation instruction per 2048 columns
  - single-partition [1,N] DVE work is minimized: c-row copies go to the
    scalar engine, the st/ts biases are folded into the activation bias
    operand so no [1,N] c+bias rows are built
"""

import numpy as np
from concourse import bacc, bass_utils, tile, mybir, bass_isa

# Semaphores fed by peer-core remote_dma writes (num, name, seed value).
# The tile scheduler's single-core CoreSim cannot model the cross-core
# increments, so _install_sched_sem_seed pre-credits them in the scheduling
# simulation only; the emitted NEFF keeps the real hardware waits.
_XCH_SEMS: list[tuple[int, str, int]] = []


def _install_sched_sem_seed():
    from concourse import bass_interp as _bi

    if getattr(tile, "_xch_seed_installed", False):
        return
    _orig = tile.CoreSim

    class _SeededCoreSim(_orig):
        def __init__(self, *a, **k):
            super().__init__(*a, **k)
            for num, name, val in _XCH_SEMS:
                self.update_semaphore(
                    mybir.SyncUpdate(
                        sync_type="semaphore", id=num, ant_name=name,
                        update_mode="sem-add-imm", update_value=val))

    tile.CoreSim = _SeededCoreSim
    tile._xch_seed_installed = True

B, N, T, D = 4, 4096, 512, 32
NT = N + T
ALPHA = 3.0
EPS = 1e-30
N_CORES = 8
RB = 2048          # ss|st rows per core
TB = 256           # ts|tt rows per core
NBLK = RB // 128   # 16 row blocks
NCH = N // 512     # 8 column chunks of 512
F32 = mybir.dt.float32
F32R = mybir.dt.float32r
BF16 = mybir.dt.bfloat16
F16 = mybir.dt.float16
TANH = mybir.ActivationFunctionType.Tanh

GL = {h: [g for g in range(N // 128) if (g % 4) // 2 == h] for h in (0, 1)}
JCS = [g // 4 for g in GL[0]]
assert JCS == [g // 4 for g in GL[1]]

N_SCAN = 18        # 72 upper-tri tiles / 4 per PSUM tile
NCOLL = 3 + N_SCAN

# packed small-weights layout (one DMA): [128, WP] f32
#  rows 0:32 -- cols 0:64  w12T  ([ss1_w.T | ss2_w.T], lhsT for uvL)
#  rows 0:32 -- cols 64:128 w21T ([ss2_w.T | ss1_w.T], lhsT for uvR)
#  rows 0:32 -- cols 128:130 wc_st2, 130:132 wc_ts2, 132:134 wa_st2,
#               134:136 wa_ts2
#  all rows  -- col 136 roff; col 137 svec (+a/-a); row 0 -- col 138 stb,
#               col 139 tsb
WP = 140


def _build_nc():
    _install_sched_sem_seed()
    nc = bacc.Bacc(trn_type="TRN2", target_bir_lowering=False, debug=False,
                   num_devices=N_CORES)

    d_in = {}
    d_in["wpack"] = nc.dram_tensor("wpack", [128, WP], F32,
                                   kind="ExternalInput")
    for name, shape in [
        ("xsrT", [D, RB]), ("xtT", [D, T]), ("xtrT", [D, TB]),
        ("xsT", [D, N]),
    ]:
        d_in[name] = nc.dram_tensor(name, shape, F32R, kind="ExternalInput")
    out_a = nc.dram_tensor("out_a", [RB, NT], F16, kind="ExternalOutput")
    out_b = nc.dram_tensor("out_b", [TB, NT], F16, kind="ExternalOutput")

    with tile.TileContext(nc) as tc:
        with tc.tile_pool(name="stg", bufs=1) as stg, \
             tc.tile_pool(name="big", bufs=1) as big, \
             tc.tile_pool(name="slabp", bufs=14) as slabp, \
             tc.tile_pool(name="sqp", bufs=2) as sqp, \
             tc.tile_pool(name="psm", bufs=2, space="PSUM") as psm:

            # ---------- input DMAs (5, most-urgent first) ----------
            wpk = stg.tile([128, WP], F32, tag="wpk")
            xsrT = stg.tile([D, RB], F32R, tag="xsrT")
            xtT = stg.tile([D, T], F32R, tag="xtT")
            xtrT = stg.tile([D, TB], F32R, tag="xtrT")
            xsT = stg.tile([D, N], F32R, tag="xsT")
            for t, d in [(wpk, "wpack"), (xsrT, "xsrT"), (xtT, "xtT"),
                         (xtrT, "xtrT"), (xsT, "xsT")]:
                nc.sync.dma_start(out=t[:], in_=d_in[d].ap())

            # ---------- cross-core exchange machinery ----------
            # One-shot all-to-all max exchange: every core broadcasts its
            # partial [128,C] to the 7 peers (XOR-relative dests, one
            # remote_dma_broadcast per slot), waits for 7 incoming buffers,
            # then max-merges locally on the (idle) gpsimd engine.  This
            # replaces collective_compute whose entry barrier + mesh walk
            # had a ~90us critical path.
            semA = nc.alloc_semaphore("xchA")
            semB = nc.alloc_semaphore("xchB")
            lsem = nc.alloc_semaphore("xchLocal")
            _XCH_SEMS.clear()
            _XCH_SEMS.append((semA.num, semA.name, 14))
            _XCH_SEMS.append((semB.num, semB.name, 14))

            def exchange_max(cur, C, sem, tagp):
                rbs = []
                for s in range(1, 8):
                    rb = big.tile([128, C], F32, tag=f"{tagp}rb{s}",
                                  name=f"{tagp}rb{s}")
                    rd = [None] * 8
                    rd[s] = (0, s)
                    nc.gpsimd.remote_dma_broadcast(
                        out_ap=rb[:], in_ap=cur[:], remote_sem=sem,
                        local_sem=lsem, rdests=rd)
                    rbs.append(rb)
                nc.gpsimd.trigger_dma(count=None)
                acc = cur
                for s, rb in enumerate(rbs):
                    nxt = big.tile([128, C], F32, tag=f"{tagp}mx{s}",
                                   name=f"{tagp}mx{s}")
                    mi = nc.vector.tensor_tensor(nxt[:], acc[:], rb[:],
                                                 mybir.AluOpType.max)
                    if s == 0:
                        mi.wait_op(sem, 14, "sem-ge")
                    acc = nxt
                return acc

            w12_f = wpk[0:D, 0:64]
            w21_f = wpk[0:D, 64:128]
            wc_st2_f = wpk[0:D, 128:130]
            wc_ts2_f = wpk[0:D, 130:132]
            wa_st2_f = wpk[0:D, 132:134]
            wa_ts2_f = wpk[0:D, 134:136]
            roff_sb = wpk[:, 136:137]
            svec = wpk[:, 137:138]      # rows 0:32 = +ALPHA, 32:64 = -ALPHA
            stb_sb = wpk[0:1, 138:139]
            tsb_sb = wpk[0:1, 139:140]
            stb_full = wpk[:, 138:139]   # bias replicated on all partitions
            tsb_full = wpk[:, 139:140]

            w12_r = big.tile([D, 64], F32R, tag="w12r")
            w21_r = big.tile([D, 64], F32R, tag="w21r")
            wc_st2_r = big.tile([D, 2], F32R, tag="wcst")
            wc_ts2_r = big.tile([D, 2], F32R, tag="wcts")
            wa_st2_r = big.tile([D, 2], F32R, tag="wast")
            wa_ts2_r = big.tile([D, 2], F32R, tag="wats")
            nc.vector.tensor_copy(w12_r[:], w12_f)
            nc.vector.tensor_copy(w21_r[:], w21_f)
            nc.vector.tensor_copy(wc_st2_r[:], wc_st2_f)
            nc.vector.tensor_copy(wc_ts2_r[:], wc_ts2_f)
            nc.vector.tensor_copy(wa_st2_r[:], wa_st2_f)
            nc.vector.tensor_copy(wa_ts2_r[:], wa_ts2_f)

            # ---------- uv builds ----------
            # uvL = [n1T_rows ; n2T_rows] x2 (bf16), uvR = [n2T ; -n1T] x2
            uvL = big.tile([128, RB], BF16, tag="uvL")
            uvR = big.tile([128, N], BF16, tag="uvR")
            pL = psm.tile([128, 2048], F32, tag="mm")
            for j in range(4):
                nc.tensor.matmul(pL[0:64, 512 * j:512 * j + 512], w12_r[:],
                                 xsrT[:, 512 * j:512 * j + 512],
                                 start=True, stop=True)
            nc.scalar.activation(uvL[0:64, :], pL[0:64, :], TANH,
                                 bias=0.0, scale=ALPHA)
            for half in range(2):
                pR = psm.tile([128, 2048], F32, tag="mm")
                for j in range(4):
                    c0 = 2048 * half + 512 * j
                    nc.tensor.matmul(pR[0:64, 512 * j:512 * j + 512],
                                     w21_r[:], xsT[:, c0:c0 + 512],
                                     start=True, stop=True)
                nc.scalar.activation(uvR[0:64, 2048 * half:2048 * half + 2048],
                                     pR[0:64, :], TANH, bias=0.0,
                                     scale=svec[0:64, 0:1])
            nc.sync.dma_start(out=uvL[64:128, :], in_=uvL[0:64, :])
            nc.sync.dma_start(out=uvR[64:128, :], in_=uvR[0:64, :])

            # ---------- c rows, a vectors, tt partials (cheap, early) ---
            # c rows land in SBUF as f32r (moving rows for the K=1 matmuls);
            # copies PSUM->SBUF happen on the (idle) scalar engine.
            c_st = big.tile([1, T], F32R, tag="cst")
            c_ts = big.tile([1, N], F32R, tag="cts")
            maxc = big.tile([1, 3], F32, tag="maxc")
            pg = psm.tile([128, 2048], F32, tag="mm")
            nc.tensor.matmul(pg[0:2, 0:512], wc_st2_r[:], xtT[:],
                             start=True, stop=True)
            nc.scalar.copy(c_st[0:1, :], pg[0:1, 0:512])
            nc.vector.tensor_reduce(maxc[0:1, 0:1], pg[0:1, 0:512],
                                    axis=mybir.AxisListType.X,
                                    op=mybir.AluOpType.max)
            for half in range(2):
                pg2 = psm.tile([128, 2048], F32, tag="mm")
                for jc in range(4):
                    nc.tensor.matmul(pg2[0:2, 512 * jc:512 * jc + 512],
                                     wc_ts2_r[:],
                                     xsT[:, 2048 * half + 512 * jc:
                                         2048 * half + 512 * jc + 512],
                                     start=True, stop=True)
                nc.scalar.copy(c_ts[0:1, 2048 * half:2048 * half + 2048],
                               pg2[0:1, 0:2048])
                nc.vector.tensor_reduce(maxc[0:1, 1 + half:2 + half],
                                        pg2[0:1, 0:2048],
                                        axis=mybir.AxisListType.X,
                                        op=mybir.AluOpType.max)
            # a vectors partition-major: a_st_pm[p, i] = a_st[128*i + p]
            a_st_pm = big.tile([128, NBLK], F32, tag="astpm")
            a_ts_pm = big.tile([128, 2], F32, tag="atspm")
            ttmaxb = big.tile([128, 2], F32, tag="ttmaxb")
            pa = psm.tile([128, 2048], F32, tag="mm")
            for i in range(NBLK):
                nc.tensor.matmul(pa[:, 2 * i:2 * i + 2],
                                 xsrT[:, 128 * i:128 * i + 128],
                                 wa_st2_r[:], start=True, stop=True)
            for m in range(2):
                nc.tensor.matmul(pa[:, 32 + 2 * m:34 + 2 * m],
                                 xtrT[:, 128 * m:128 * m + 128],
                                 wa_ts2_r[:], start=True, stop=True)
            nc.vector.tensor_copy(
                a_st_pm[:], pa[:, 0:32].rearrange("p (n s) -> p n s", s=2)[:, :, 0:1])
            nc.vector.tensor_copy(
                a_ts_pm[:], pa[:, 32:36].rearrange("p (n s) -> p n s", s=2)[:, :, 0:1])
            ptt = psm.tile([128, 2048], F32, tag="mm")
            for m in range(2):
                nc.tensor.matmul(ptt[:, 512 * m:512 * m + 512],
                                 xtrT[:, 128 * m:128 * m + 128],
                                 xtT[:], start=True, stop=True)
                nc.vector.tensor_reduce(ttmaxb[:, m:m + 1],
                                        ptt[:, 512 * m:512 * m + 512],
                                        axis=mybir.AxisListType.X,
                                        op=mybir.AluOpType.max)

            # ---------- partial maxes + collective A (st/ts/tt) ----------
            # part[:, 0] = st max, 1 = ts max, 2 = tt max, 3:3+18 = ss tiles
            part = big.tile([128, NCOLL], F32, tag="part")
            nc.vector.memset(part[:, 0:3], 0.0)
            maxa_st = big.tile([128, 1], F32, tag="maxast")
            maxa_ts = big.tile([128, 1], F32, tag="maxats")
            nc.vector.tensor_reduce(maxa_st[:], a_st_pm[:],
                                    axis=mybir.AxisListType.X,
                                    op=mybir.AluOpType.max)
            nc.vector.tensor_reduce(maxa_ts[:], a_ts_pm[:],
                                    axis=mybir.AxisListType.X,
                                    op=mybir.AluOpType.max)
            maxa_st_r = big.tile([128, 1], F32, tag="maxastr")
            maxa_ts_r = big.tile([128, 1], F32, tag="maxatsr")
            nc.gpsimd.partition_all_reduce(maxa_st_r[:], maxa_st[:],
                                           channels=128,
                                           reduce_op=bass_isa.ReduceOp.max)
            nc.gpsimd.partition_all_reduce(maxa_ts_r[:], maxa_ts[:],
                                           channels=128,
                                           reduce_op=bass_isa.ReduceOp.max)
            maxc_ts = big.tile([1, 1], F32, tag="maxcts2")
            nc.vector.tensor_reduce(maxc_ts[:], maxc[0:1, 1:3],
                                    axis=mybir.AxisListType.X,
                                    op=mybir.AluOpType.max)
            tmp_st = big.tile([1, 1], F32, tag="tmpst")
            tmp_ts = big.tile([1, 1], F32, tag="tmpts")
            nc.vector.tensor_tensor(tmp_st[:], maxa_st_r[0:1, 0:1],
                                    maxc[0:1, 0:1], mybir.AluOpType.add)
            nc.vector.tensor_tensor(part[0:1, 0:1], tmp_st[:], stb_sb,
                                    mybir.AluOpType.add)
            nc.vector.tensor_tensor(tmp_ts[:], maxa_ts_r[0:1, 0:1],
                                    maxc_ts[:], mybir.AluOpType.add)
            nc.vector.tensor_tensor(part[0:1, 1:2], tmp_ts[:], tsb_sb,
                                    mybir.AluOpType.add)
            nc.vector.tensor_reduce(part[:, 2:3], ttmaxb[:],
                                    axis=mybir.AxisListType.X,
                                    op=mybir.AluOpType.max)
            nc.vector.tensor_scalar_max(part[:, 0:3], part[:, 0:3], 0.0)
            partA_r = big.tile([128, 3], F32, tag="partAr")
            nc.gpsimd.partition_all_reduce(partA_r[:], part[:, 0:3],
                                           channels=128,
                                           reduce_op=bass_isa.ReduceOp.max)
            gmaxA = exchange_max(partA_r, 3, semA, "A")

            # ---------- ss abs-max scan (fills part cols 3:3+18) ----------
            tiles1 = [(i, jc) for i in range(NBLK)
                      for jc in range(JCS[i], NCH)]
            assert len(tiles1) == 4 * N_SCAN
            ABS = mybir.ActivationFunctionType.Abs
            for t in range(N_SCAN):
                pm_ = psm.tile([128, 2048], F32, tag="mm")
                for s in range(4):
                    i, jc = tiles1[4 * t + s]
                    q = 64 * (s % 2)
                    nc.tensor.matmul(pm_[:, 512 * s:512 * s + 512],
                                     uvL[q:q + 64, 128 * i:128 * i + 128],
                                     uvR[q:q + 64, 512 * jc:512 * jc + 512],
                                     start=True, stop=True,
                                     tile_position=(q, 0))
                if t % 2 == 1:
                    # offload the |.| pass to the scalar engine (idle during
                    # the scan); DVE then max-reduces bf16 from SBUF
                    ab = sqp.tile([128, 2048], BF16, tag="sq")
                    nc.scalar.activation(ab[:], pm_[:], ABS)
                    nc.vector.tensor_reduce(part[:, 3 + t:4 + t], ab[:],
                                            axis=mybir.AxisListType.X,
                                            op=mybir.AluOpType.max)
                else:
                    nc.vector.tensor_reduce(part[:, 3 + t:4 + t], pm_[:],
                                            axis=mybir.AxisListType.X,
                                            op=mybir.AluOpType.max,
                                            apply_absolute_value=True)

            # ---------- exchange B (ss scan maxes) ----------
            partB_r = big.tile([128, N_SCAN], F32, tag="partBr")
            nc.gpsimd.partition_all_reduce(partB_r[:], part[:, 3:NCOLL],
                                           channels=128,
                                           reduce_op=bass_isa.ReduceOp.max)
            gmaxB = exchange_max(partB_r, N_SCAN, semB, "B")

            # ---------- consume exchange A ----------
            t3 = big.tile([128, 3], F32, tag="t3")
            nc.vector.tensor_scalar_add(t3[:], gmaxA[:], EPS)
            scales3 = big.tile([128, 3], F32, tag="scales3")
            nc.vector.reciprocal(scales3[:], t3[:])
            # biases folded: sab_st[:, i] = (a_st + stb) * scale_st
            sab_st = big.tile([128, NBLK], F32, tag="sabst")
            sab_ts = big.tile([128, 2], F32, tag="sabts")
            nc.vector.tensor_scalar(sab_st[:], a_st_pm[:], stb_full,
                                    scales3[:, 0:1], mybir.AluOpType.add,
                                    mybir.AluOpType.mult)
            nc.vector.tensor_scalar(sab_ts[:], a_ts_pm[:], tsb_full,
                                    scales3[:, 1:2], mybir.AluOpType.add,
                                    mybir.AluOpType.mult)

            ones_lhsT = big.tile([1, 128], F32R, tag="ones")
            nc.vector.tensor_scalar(ones_lhsT[:], xsT[0:1, 0:128], 0.0, 1.0,
                                    mybir.AluOpType.mult, mybir.AluOpType.add)

            # tt triu masks (fp16)
            msks = []
            for m in range(2):
                itF = big.tile([128, 512], F32, tag=f"itF{m}")
                nc.gpsimd.iota(itF[:], pattern=[[1, 512]], base=-128 * m,
                               channel_multiplier=-1,
                               allow_small_or_imprecise_dtypes=True)
                msk = big.tile([128, 512], F16, tag=f"msk{m}")
                nc.vector.tensor_scalar(msk[:], itF[:], roff_sb, None,
                                        mybir.AluOpType.is_ge)
                msks.append(msk)

            # ---------- pass 2B: [ts | tt] (gated on collective A) ----------
            for m in range(2):
                slab = slabp.tile([128, NT], F16, tag="slab",
                                  name=f"slabB{m}")
                for half in range(2):
                    pm_ = psm.tile([128, 2048], F32, tag="mm")
                    for j in range(4):
                        c0 = 2048 * half + 512 * j
                        nc.tensor.matmul(pm_[:, 512 * j:512 * j + 512],
                                         ones_lhsT[:],
                                         c_ts[0:1, c0:c0 + 512],
                                         start=True, stop=True)
                    nc.scalar.activation(
                        slab[:, 2048 * half:2048 * half + 2048], pm_[:],
                        TANH, bias=sab_ts[:, m:m + 1], scale=scales3[:, 1:2])
                pm_ = psm.tile([128, 2048], F32, tag="mm")
                nc.tensor.matmul(pm_[:, 0:512],
                                 xtrT[:, 128 * m:128 * m + 128],
                                 xtT[:], start=True, stop=True)
                nc.scalar.activation(slab[:, N:NT], pm_[:, 0:512], TANH,
                                     bias=0.0, scale=scales3[:, 2:3])
                nc.vector.tensor_scalar_max(slab[:], slab[:], 0.0)
                nc.vector.tensor_tensor(slab[:, N:NT], slab[:, N:NT],
                                        msks[m][:], mybir.AluOpType.mult)
                nc.sync.dma_start(out=out_b.ap()[128 * m:128 * m + 128, :],
                                  in_=slab[:])

            # ---------- st prefill for first slabs (collective-A gated) ----
            # Fills the [st] columns of slabs 0..PRE-1 while collective B is
            # in flight; their [ss] columns are written post-collective-B.
            PRE = 12
            slabs = [slabp.tile([128, NT], F16, tag="slab", name=f"slab{k}")
                     for k in range(PRE)]

            def st_fill(slab, i):
                pm_ = psm.tile([128, 2048], F32, tag="mm")
                nc.tensor.matmul(pm_[:, 0:512], ones_lhsT[:], c_st[:],
                                 start=True, stop=True)
                nc.scalar.activation(slab[:, N:NT], pm_[:, 0:512], TANH,
                                     bias=sab_st[:, i:i + 1],
                                     scale=scales3[:, 0:1])

            for i in range(PRE):
                st_fill(slabs[i], i)

            # ---------- consume exchange B ----------
            gss = big.tile([128, 1], F32, tag="gss")
            nc.vector.tensor_reduce(gss[:], gmaxB[:],
                                    axis=mybir.AxisListType.X,
                                    op=mybir.AluOpType.max)
            t1 = big.tile([128, 1], F32, tag="t1")
            nc.vector.tensor_scalar(t1[:], gss[:], ALPHA, EPS,
                                    mybir.AluOpType.mult,
                                    mybir.AluOpType.add)
            rec1 = big.tile([128, 1], F32, tag="rec1")
            nc.vector.reciprocal(rec1[:], t1[:])
            s_ss = big.tile([128, 1], F32, tag="sss")
            nc.vector.tensor_scalar_mul(s_ss[:], rec1[:], ALPHA)

            # ---------- main pass: [ss | st] slabs ----------
            for i in range(NBLK):
                if i < PRE:
                    slab = slabs[i]
                else:
                    slab = slabp.tile([128, NT], F16, tag="slab",
                                      name=f"slabA{i}")
                    st_fill(slab, i)
                for half in range(2):
                    pm_ = psm.tile([128, 2048], F32, tag="mm")
                    for s in range(4):
                        jc = 4 * half + s
                        q = 64 * (s % 2)
                        nc.tensor.matmul(pm_[:, 512 * s:512 * s + 512],
                                         uvL[q:q + 64, 128 * i:128 * i + 128],
                                         uvR[q:q + 64, 512 * jc:512 * jc + 512],
                                         start=True, stop=True,
                                         tile_position=(q, 0))
                    nc.scalar.activation(
                        slab[:, 2048 * half:2048 * half + 2048], pm_[:],
                        TANH, bias=0.0, scale=s_ss[:, 0:1])
                nc.vector.tensor_scalar_max(slab[:], slab[:], 0.0)
                nc.sync.dma_start(out=out_a.ap()[128 * i:128 * i + 128, :],
                                  in_=slab[:])

    nc.finalize()
    return nc


def _in_maps(spatial_nodes, temporal_nodes, ss1_w, ss2_w, st_w, st_b, ts_w, ts_b):
    f = np.float32
    maps = []
    wpack = np.zeros((128, WP), dtype=f)
    wpack[0:D, 0:D] = ss1_w.T
    wpack[0:D, D:2 * D] = ss2_w.T
    wpack[0:D, 64:64 + D] = ss2_w.T
    wpack[0:D, 64 + D:128] = ss1_w.T
    wpack[0:D, 128:130] = np.stack([st_w[0, D:], st_w[0, D:]], 1)
    wpack[0:D, 130:132] = np.stack([ts_w[0, D:], ts_w[0, D:]], 1)
    wpack[0:D, 132:134] = np.stack([st_w[0, :D], st_w[0, :D]], 1)
    wpack[0:D, 134:136] = np.stack([ts_w[0, :D], ts_w[0, :D]], 1)
    wpack[0:D, 137] = ALPHA
    wpack[D:2 * D, 137] = -ALPHA
    wpack[:, 138] = np.float32(np.asarray(st_b).reshape(-1)[0])
    wpack[:, 139] = np.float32(np.asarray(ts_b).reshape(-1)[0])
    for c in range(N_CORES):
        b, h = c // 2, c % 2
        wp = wpack.copy()
        wp[:, 136] = TB * h
        xs_b = np.asarray(spatial_nodes[b], dtype=f)
        xt_b = np.asarray(temporal_nodes[b], dtype=f)
        xs_rows = np.concatenate(
            [xs_b[128 * g:128 * g + 128] for g in GL[h]], 0)
        maps.append({
            "xsT": np.ascontiguousarray(xs_b.T),
            "xsrT": np.ascontiguousarray(xs_rows.T),
            "xtT": np.ascontiguousarray(xt_b.T),
            "xtrT": np.ascontiguousarray(xt_b[TB * h:TB * h + TB].T),
            "wpack": wp,
        })
    return maps


def run_kernel(inputs, trace=False, **spmd_kwargs):
    nc = _build_nc()
    maps = _in_maps(**inputs)
    res = bass_utils.run_bass_kernel_spmd(
        nc, maps, core_ids=list(range(N_CORES)), trace=trace, **spmd_kwargs)
    adj = np.empty((B, NT, NT), dtype=np.float32)
    for c in range(N_CORES):
        b, h = c // 2, c % 2
        oa = np.asarray(res.results[c]["out_a"], dtype=np.float32)
        ob = np.asarray(res.results[c]["out_b"], dtype=np.float32)
        for li, g in enumerate(GL[h]):
            adj[b, 128 * g:128 * g + 128, :] = oa[128 * li:128 * li + 128]
        adj[b, N + TB * h:N + TB * h + TB, :] = ob
    return adj, res


def kernel(**inputs):
    adj, _ = run_kernel(inputs, trace=False)
    return adj


# revision 33
# speedup vs baseline: 48.9302x; 48.9302x over previous
"""Trainium2 Bass kernel for nn_MLPSimDirectNormConstructor (gnn adjacency builder).

adj = [uni_adj(ss) | uni_adj(st); uni_adj(ts) | triu(uni_adj(tt))] for
  spatial_nodes [4,4096,32], temporal_nodes [4,512,32].

Sharding: 8 cores = (batch b = c//2, half h = c%2).  Each core produces
  - 16 interleaved 128-row blocks of the [ss|st] region (rows 128g, g in GL[h])
  - 256 rows of the [ts|tt] region (rows h*256 .. h*256+256)
The interleaved row-block assignment (g%4 in {2h,2h+1}) makes the
upper-triangle-only abs-max scan of the antisymmetric ss block both
load-balanced and SPMD-uniform.

v3 structure:
  - host does all x transposes (layout prep only)
  - ONE AllReduce(max) of a [128,16] buffer carrying the st/ts/tt maxes
    (cols 0:3) and the 13 per-tile ss abs-maxes (cols 3:16); the ss global
    max is reduced from the collective result.  (v2 used two serialized
    collectives; the second one's trigger+mesh cost ~35us of exposed
    latency.)
  - outputs fp16, uv factors bf16, PSUM as 2x[128,2048], one tanh
    activation instruction per 2048 columns
  - single-partition [1,N] DVE work is minimized: c-row copies go to the
    scalar engine, the st/ts biases are folded into the activation bias
    operand so no [1,N] c+bias rows are built
"""

import numpy as np
from concourse import bacc, bass_utils, tile, mybir, bass_isa

B, N, T, D = 4, 4096, 512, 32
NT = N + T
ALPHA = 3.0
EPS = 1e-30
N_CORES = 8
RB = 2048          # ss|st rows per core
TB = 256           # ts|tt rows per core
NBLK = RB // 128   # 16 row blocks
NCH = N // 512     # 8 column chunks of 512
F32 = mybir.dt.float32
F32R = mybir.dt.float32r
BF16 = mybir.dt.bfloat16
F16 = mybir.dt.float16
TANH = mybir.ActivationFunctionType.Tanh

GL = {h: [g for g in range(N // 128) if (g % 4) // 2 == h] for h in (0, 1)}
JCS = [g // 4 for g in GL[0]]
assert JCS == [g // 4 for g in GL[1]]

N_SCAN = 18        # 72 upper-tri tiles / 4 per PSUM tile
NCOLL = 3 + N_SCAN

# packed small-weights layout (one DMA): [128, WP] f32
#  rows 0:32 -- cols 0:64  w12T  ([ss1_w.T | ss2_w.T], lhsT for uvL)
#  rows 0:32 -- cols 64:128 w21T ([ss2_w.T | ss1_w.T], lhsT for uvR)
#  rows 0:32 -- cols 128:130 wc_st2, 130:132 wc_ts2, 132:134 wa_st2,
#               134:136 wa_ts2
#  all rows  -- col 136 roff; col 137 svec (+a/-a); row 0 -- col 138 stb,
#               col 139 tsb
WP = 140


def _build_nc():
    nc = bacc.Bacc(trn_type="TRN2", target_bir_lowering=False, debug=False,
                   num_devices=N_CORES)

    d_in = {}
    d_in["wpack"] = nc.dram_tensor("wpack", [128, WP], F32,
                                   kind="ExternalInput")
    for name, shape in [
        ("xsrT", [D, RB]), ("xtT", [D, T]), ("xtrT", [D, TB]),
        ("xsT", [D, N]),
    ]:
        d_in[name] = nc.dram_tensor(name, shape, F32R, kind="ExternalInput")
    out_a = nc.dram_tensor("out_a", [RB, NT], F16, kind="ExternalOutput")
    out_b = nc.dram_tensor("out_b", [TB, NT], F16, kind="ExternalOutput")

    with tile.TileContext(nc) as tc:
        with tc.tile_pool(name="stg", bufs=1) as stg, \
             tc.tile_pool(name="big", bufs=1) as big, \
             tc.tile_pool(name="slabp", bufs=10) as slabp, \
             tc.tile_pool(name="psm", bufs=2, space="PSUM") as psm, \
             tc.tile_pool(name="drm", bufs=1, space="DRAM") as drm:

            # ---------- input DMAs (5, most-urgent first) ----------
            wpk = stg.tile([128, WP], F32, tag="wpk")
            xsrT = stg.tile([D, RB], F32R, tag="xsrT")
            xtT = stg.tile([D, T], F32R, tag="xtT")
            xtrT = stg.tile([D, TB], F32R, tag="xtrT")
            xsT = stg.tile([D, N], F32R, tag="xsT")
            for t, d in [(wpk, "wpack"), (xsrT, "xsrT"), (xtT, "xtT"),
                         (xtrT, "xtrT"), (xsT, "xsT")]:
                nc.sync.dma_start(out=t[:], in_=d_in[d].ap())

            w12_f = wpk[0:D, 0:64]
            w21_f = wpk[0:D, 64:128]
            wc_st2_f = wpk[0:D, 128:130]
            wc_ts2_f = wpk[0:D, 130:132]
            wa_st2_f = wpk[0:D, 132:134]
            wa_ts2_f = wpk[0:D, 134:136]
            roff_sb = wpk[:, 136:137]
            svec = wpk[:, 137:138]      # rows 0:32 = +ALPHA, 32:64 = -ALPHA
            stb_sb = wpk[0:1, 138:139]
            tsb_sb = wpk[0:1, 139:140]
            stb_full = wpk[:, 138:139]   # bias replicated on all partitions
            tsb_full = wpk[:, 139:140]

            w12_r = big.tile([D, 64], F32R, tag="w12r")
            w21_r = big.tile([D, 64], F32R, tag="w21r")
            wc_st2_r = big.tile([D, 2], F32R, tag="wcst")
            wc_ts2_r = big.tile([D, 2], F32R, tag="wcts")
            wa_st2_r = big.tile([D, 2], F32R, tag="wast")
            wa_ts2_r = big.tile([D, 2], F32R, tag="wats")
            nc.vector.tensor_copy(w12_r[:], w12_f)
            nc.vector.tensor_copy(w21_r[:], w21_f)
            nc.vector.tensor_copy(wc_st2_r[:], wc_st2_f)
            nc.vector.tensor_copy(wc_ts2_r[:], wc_ts2_f)
            nc.vector.tensor_copy(wa_st2_r[:], wa_st2_f)
            nc.vector.tensor_copy(wa_ts2_r[:], wa_ts2_f)

            # ---------- uv builds ----------
            # uvL = [n1T_rows ; n2T_rows] x2 (bf16), uvR = [n2T ; -n1T] x2
            uvL = big.tile([128, RB], BF16, tag="uvL")
            uvR = big.tile([128, N], BF16, tag="uvR")
            pL = psm.tile([128, 2048], F32, tag="mm")
            for j in range(4):
                nc.tensor.matmul(pL[0:64, 512 * j:512 * j + 512], w12_r[:],
                                 xsrT[:, 512 * j:512 * j + 512],
                                 start=True, stop=True)
            nc.scalar.activation(uvL[0:64, :], pL[0:64, :], TANH,
                                 bias=0.0, scale=ALPHA)
            for half in range(2):
                pR = psm.tile([128, 2048], F32, tag="mm")
                for j in range(4):
                    c0 = 2048 * half + 512 * j
                    nc.tensor.matmul(pR[0:64, 512 * j:512 * j + 512],
                                     w21_r[:], xsT[:, c0:c0 + 512],
                                     start=True, stop=True)
                nc.scalar.activation(uvR[0:64, 2048 * half:2048 * half + 2048],
                                     pR[0:64, :], TANH, bias=0.0,
                                     scale=svec[0:64, 0:1])
            nc.sync.dma_start(out=uvL[64:128, :], in_=uvL[0:64, :])
            nc.sync.dma_start(out=uvR[64:128, :], in_=uvR[0:64, :])

            # ---------- c rows, a vectors, tt partials (cheap, early) ---
            # c rows land in SBUF as f32r (moving rows for the K=1 matmuls);
            # copies PSUM->SBUF happen on the (idle) scalar engine.
            c_st = big.tile([1, T], F32R, tag="cst")
            c_ts = big.tile([1, N], F32R, tag="cts")
            maxc = big.tile([1, 3], F32, tag="maxc")
            pg = psm.tile([128, 2048], F32, tag="mm")
            nc.tensor.matmul(pg[0:2, 0:512], wc_st2_r[:], xtT[:],
                             start=True, stop=True)
            nc.scalar.copy(c_st[0:1, :], pg[0:1, 0:512])
            nc.vector.tensor_reduce(maxc[0:1, 0:1], pg[0:1, 0:512],
                                    axis=mybir.AxisListType.X,
                                    op=mybir.AluOpType.max)
            for half in range(2):
                pg2 = psm.tile([128, 2048], F32, tag="mm")
                for jc in range(4):
                    nc.tensor.matmul(pg2[0:2, 512 * jc:512 * jc + 512],
                                     wc_ts2_r[:],
                                     xsT[:, 2048 * half + 512 * jc:
                                         2048 * half + 512 * jc + 512],
                                     start=True, stop=True)
                nc.scalar.copy(c_ts[0:1, 2048 * half:2048 * half + 2048],
                               pg2[0:1, 0:2048])
                nc.vector.tensor_reduce(maxc[0:1, 1 + half:2 + half],
                                        pg2[0:1, 0:2048],
                                        axis=mybir.AxisListType.X,
                                        op=mybir.AluOpType.max)
            # a vectors partition-major: a_st_pm[p, i] = a_st[128*i + p]
            a_st_pm = big.tile([128, NBLK], F32, tag="astpm")
            a_ts_pm = big.tile([128, 2], F32, tag="atspm")
            ttmaxb = big.tile([128, 2], F32, tag="ttmaxb")
            pa = psm.tile([128, 2048], F32, tag="mm")
            for i in range(NBLK):
                nc.tensor.matmul(pa[:, 2 * i:2 * i + 2],
                                 xsrT[:, 128 * i:128 * i + 128],
                                 wa_st2_r[:], start=True, stop=True)
            for m in range(2):
                nc.tensor.matmul(pa[:, 32 + 2 * m:34 + 2 * m],
                                 xtrT[:, 128 * m:128 * m + 128],
                                 wa_ts2_r[:], start=True, stop=True)
            nc.vector.tensor_copy(
                a_st_pm[:], pa[:, 0:32].rearrange("p (n s) -> p n s", s=2)[:, :, 0:1])
            nc.vector.tensor_copy(
                a_ts_pm[:], pa[:, 32:36].rearrange("p (n s) -> p n s", s=2)[:, :, 0:1])
            ptt = psm.tile([128, 2048], F32, tag="mm")
            for m in range(2):
                nc.tensor.matmul(ptt[:, 512 * m:512 * m + 512],
                                 xtrT[:, 128 * m:128 * m + 128],
                                 xtT[:], start=True, stop=True)
                nc.vector.tensor_reduce(ttmaxb[:, m:m + 1],
                                        ptt[:, 512 * m:512 * m + 512],
                                        axis=mybir.AxisListType.X,
                                        op=mybir.AluOpType.max)

            # ---------- partial maxes + collective A (st/ts/tt) ----------
            # part[:, 0] = st max, 1 = ts max, 2 = tt max, 3:3+18 = ss tiles
            part = big.tile([128, NCOLL], F32, tag="part")
            nc.vector.memset(part[:, 0:3], 0.0)
            maxa_st = big.tile([128, 1], F32, tag="maxast")
            maxa_ts = big.tile([128, 1], F32, tag="maxats")
            nc.vector.tensor_reduce(maxa_st[:], a_st_pm[:],
                                    axis=mybir.AxisListType.X,
                                    op=mybir.AluOpType.max)
            nc.vector.tensor_reduce(maxa_ts[:], a_ts_pm[:],
                                    axis=mybir.AxisListType.X,
                                    op=mybir.AluOpType.max)
            maxa_st_r = big.tile([128, 1], F32, tag="maxastr")
            maxa_ts_r = big.tile([128, 1], F32, tag="maxatsr")
            nc.gpsimd.partition_all_reduce(maxa_st_r[:], maxa_st[:],
                                           channels=128,
                                           reduce_op=bass_isa.ReduceOp.max)
            nc.gpsimd.partition_all_reduce(maxa_ts_r[:], maxa_ts[:],
                                           channels=128,
                                           reduce_op=bass_isa.ReduceOp.max)
            maxc_ts = big.tile([1, 1], F32, tag="maxcts2")
            nc.vector.tensor_reduce(maxc_ts[:], maxc[0:1, 1:3],
                                    axis=mybir.AxisListType.X,
                                    op=mybir.AluOpType.max)
            tmp_st = big.tile([1, 1], F32, tag="tmpst")
            tmp_ts = big.tile([1, 1], F32, tag="tmpts")
            nc.vector.tensor_tensor(tmp_st[:], maxa_st_r[0:1, 0:1],
                                    maxc[0:1, 0:1], mybir.AluOpType.add)
            nc.vector.tensor_tensor(part[0:1, 0:1], tmp_st[:], stb_sb,
                                    mybir.AluOpType.add)
            nc.vector.tensor_tensor(tmp_ts[:], maxa_ts_r[0:1, 0:1],
                                    maxc_ts[:], mybir.AluOpType.add)
            nc.vector.tensor_tensor(part[0:1, 1:2], tmp_ts[:], tsb_sb,
                                    mybir.AluOpType.add)
            nc.vector.tensor_reduce(part[:, 2:3], ttmaxb[:],
                                    axis=mybir.AxisListType.X,
                                    op=mybir.AluOpType.max)
            nc.vector.tensor_scalar_max(part[:, 0:3], part[:, 0:3], 0.0)
            partA_r = big.tile([128, 3], F32, tag="partAr")
            nc.gpsimd.partition_all_reduce(partA_r[:], part[:, 0:3],
                                           channels=128,
                                           reduce_op=bass_isa.ReduceOp.max)
            binA = drm.tile([128, 3], F32, tag="binA")
            boutA = drm.tile([128, 3], F32, tag="boutA")
            nc.sync.dma_start(out=binA[:], in_=partA_r[:])
            nc.gpsimd.collective_compute(
                "AllReduce", mybir.AluOpType.max,
                replica_groups=[list(range(N_CORES))],
                ins=[binA.opt()], outs=[boutA.opt()])

            # ---------- ss abs-max scan (fills part cols 3:3+18) ----------
            tiles1 = [(i, jc) for i in range(NBLK)
                      for jc in range(JCS[i], NCH)]
            assert len(tiles1) == 4 * N_SCAN
            for t in range(N_SCAN):
                pm_ = psm.tile([128, 2048], F32, tag="mm")
                for s in range(4):
                    i, jc = tiles1[4 * t + s]
                    q = 64 * (s % 2)
                    nc.tensor.matmul(pm_[:, 512 * s:512 * s + 512],
                                     uvL[q:q + 64, 128 * i:128 * i + 128],
                                     uvR[q:q + 64, 512 * jc:512 * jc + 512],
                                     start=True, stop=True,
                                     tile_position=(q, 0))
                nc.vector.tensor_reduce(part[:, 3 + t:4 + t], pm_[:],
                                        axis=mybir.AxisListType.X,
                                        op=mybir.AluOpType.max,
                                        apply_absolute_value=True)

            # ---------- collective B (ss scan maxes) ----------
            partB_r = big.tile([128, N_SCAN], F32, tag="partBr")
            nc.gpsimd.partition_all_reduce(partB_r[:], part[:, 3:NCOLL],
                                           channels=128,
                                           reduce_op=bass_isa.ReduceOp.max)
            binB = drm.tile([128, N_SCAN], F32, tag="binB")
            boutB = drm.tile([128, N_SCAN], F32, tag="boutB")
            nc.sync.dma_start(out=binB[:], in_=partB_r[:])
            nc.gpsimd.collective_compute(
                "AllReduce", mybir.AluOpType.max,
                replica_groups=[list(range(N_CORES))],
                ins=[binB.opt()], outs=[boutB.opt()])

            # ---------- consume collective A ----------
            gmaxA = big.tile([128, 3], F32, tag="gmaxA")
            nc.sync.dma_start(out=gmaxA[:], in_=boutA[:])
            t3 = big.tile([128, 3], F32, tag="t3")
            nc.vector.tensor_scalar_add(t3[:], gmaxA[:], EPS)
            scales3 = big.tile([128, 3], F32, tag="scales3")
            nc.vector.reciprocal(scales3[:], t3[:])
            # biases folded: sab_st[:, i] = (a_st + stb) * scale_st
            sab_st = big.tile([128, NBLK], F32, tag="sabst")
            sab_ts = big.tile([128, 2], F32, tag="sabts")
            nc.vector.tensor_scalar(sab_st[:], a_st_pm[:], stb_full,
                                    scales3[:, 0:1], mybir.AluOpType.add,
                                    mybir.AluOpType.mult)
            nc.vector.tensor_scalar(sab_ts[:], a_ts_pm[:], tsb_full,
                                    scales3[:, 1:2], mybir.AluOpType.add,
                                    mybir.AluOpType.mult)

            ones_lhsT = big.tile([1, 128], F32R, tag="ones")
            nc.vector.tensor_scalar(ones_lhsT[:], xsT[0:1, 0:128], 0.0, 1.0,
                                    mybir.AluOpType.mult, mybir.AluOpType.add)

            # tt triu masks (fp16)
            msks = []
            for m in range(2):
                itF = big.tile([128, 512], F32, tag=f"itF{m}")
                nc.gpsimd.iota(itF[:], pattern=[[1, 512]], base=-128 * m,
                               channel_multiplier=-1,
                               allow_small_or_imprecise_dtypes=True)
                msk = big.tile([128, 512], F16, tag=f"msk{m}")
                nc.vector.tensor_scalar(msk[:], itF[:], roff_sb, None,
                                        mybir.AluOpType.is_ge)
                msks.append(msk)

            # ---------- pass 2B: [ts | tt] (gated on collective A) ----------
            for m in range(2):
                slab = slabp.tile([128, NT], F16, tag="slab",
                                  name=f"slabB{m}")
                for half in range(2):
                    pm_ = psm.tile([128, 2048], F32, tag="mm")
                    for j in range(4):
                        c0 = 2048 * half + 512 * j
                        nc.tensor.matmul(pm_[:, 512 * j:512 * j + 512],
                                         ones_lhsT[:],
                                         c_ts[0:1, c0:c0 + 512],
                                         start=True, stop=True)
                    nc.scalar.activation(
                        slab[:, 2048 * half:2048 * half + 2048], pm_[:],
                        TANH, bias=sab_ts[:, m:m + 1], scale=scales3[:, 1:2])
                pm_ = psm.tile([128, 2048], F32, tag="mm")
                nc.tensor.matmul(pm_[:, 0:512],
                                 xtrT[:, 128 * m:128 * m + 128],
                                 xtT[:], start=True, stop=True)
                nc.scalar.activation(slab[:, N:NT], pm_[:, 0:512], TANH,
                                     bias=0.0, scale=scales3[:, 2:3])
                nc.vector.tensor_scalar_max(slab[:], slab[:], 0.0)
                nc.vector.tensor_tensor(slab[:, N:NT], slab[:, N:NT],
                                        msks[m][:], mybir.AluOpType.mult)
                nc.sync.dma_start(out=out_b.ap()[128 * m:128 * m + 128, :],
                                  in_=slab[:])

            # ---------- st prefill for first slabs (collective-A gated) ----
            # Fills the [st] columns of slabs 0..PRE-1 while collective B is
            # in flight; their [ss] columns are written post-collective-B.
            PRE = 8
            slabs = [slabp.tile([128, NT], F16, tag="slab", name=f"slab{k}")
                     for k in range(PRE)]

            def st_fill(slab, i):
                pm_ = psm.tile([128, 2048], F32, tag="mm")
                nc.tensor.matmul(pm_[:, 0:512], ones_lhsT[:], c_st[:],
                                 start=True, stop=True)
                nc.scalar.activation(slab[:, N:NT], pm_[:, 0:512], TANH,
                                     bias=sab_st[:, i:i + 1],
                                     scale=scales3[:, 0:1])

            for i in range(PRE):
                st_fill(slabs[i], i)

            # ---------- consume collective B ----------
            gmaxB = big.tile([128, N_SCAN], F32, tag="gmaxB")
            nc.sync.dma_start(out=gmaxB[:], in_=boutB[:])
            gss = big.tile([128, 1], F32, tag="gss")
            nc.vector.tensor_reduce(gss[:], gmaxB[:],
                                    axis=mybir.AxisListType.X,
                                    op=mybir.AluOpType.max)
            t1 = big.tile([128, 1], F32, tag="t1")
            nc.vector.tensor_scalar(t1[:], gss[:], ALPHA, EPS,
                                    mybir.AluOpType.mult,
                                    mybir.AluOpType.add)
            rec1 = big.tile([128, 1], F32, tag="rec1")
            nc.vector.reciprocal(rec1[:], t1[:])
            s_ss = big.tile([128, 1], F32, tag="sss")
            nc.vector.tensor_scalar_mul(s_ss[:], rec1[:], ALPHA)

            # ---------- main pass: [ss | st] slabs ----------
            for i in range(NBLK):
                if i < PRE:
                    slab = slabs[i]
                else:
                    slab = slabp.tile([128, NT], F16, tag="slab",
                                      name=f"slabA{i}")
                    st_fill(slab, i)
                for half in range(2):
                    pm_ = psm.tile([128, 2048], F32, tag="mm")
                    for s in range(4):
                        jc = 4 * half + s
                        q = 64 * (s % 2)
                        nc.tensor.matmul(pm_[:, 512 * s:512 * s + 512],
                                         uvL[q:q + 64, 128 * i:128 * i + 128],
                                         uvR[q:q + 64, 512 * jc:512 * jc + 512],
                                         start=True, stop=True,
                                         tile_position=(q, 0))
                    nc.scalar.activation(
                        slab[:, 2048 * half:2048 * half + 2048], pm_[:],
                        TANH, bias=0.0, scale=s_ss[:, 0:1])
                nc.vector.tensor_scalar_max(slab[:], slab[:], 0.0)
                nc.sync.dma_start(out=out_a.ap()[128 * i:128 * i + 128, :],
                                  in_=slab[:])

    nc.finalize()
    return nc


def _in_maps(spatial_nodes, temporal_nodes, ss1_w, ss2_w, st_w, st_b, ts_w, ts_b):
    f = np.float32
    maps = []
    wpack = np.zeros((128, WP), dtype=f)
    wpack[0:D, 0:D] = ss1_w.T
    wpack[0:D, D:2 * D] = ss2_w.T
    wpack[0:D, 64:64 + D] = ss2_w.T
    wpack[0:D, 64 + D:128] = ss1_w.T
    wpack[0:D, 128:130] = np.stack([st_w[0, D:], st_w[0, D:]], 1)
    wpack[0:D, 130:132] = np.stack([ts_w[0, D:], ts_w[0, D:]], 1)
    wpack[0:D, 132:134] = np.stack([st_w[0, :D], st_w[0, :D]], 1)
    wpack[0:D, 134:136] = np.stack([ts_w[0, :D], ts_w[0, :D]], 1)
    wpack[0:D, 137] = ALPHA
    wpack[D:2 * D, 137] = -ALPHA
    wpack[:, 138] = np.float32(np.asarray(st_b).reshape(-1)[0])
    wpack[:, 139] = np.float32(np.asarray(ts_b).reshape(-1)[0])
    for c in range(N_CORES):
        b, h = c // 2, c % 2
        wp = wpack.copy()
        wp[:, 136] = TB * h
        xs_b = np.asarray(spatial_nodes[b], dtype=f)
        xt_b = np.asarray(temporal_nodes[b], dtype=f)
        xs_rows = np.concatenate(
            [xs_b[128 * g:128 * g + 128] for g in GL[h]], 0)
        maps.append({
            "xsT": np.ascontiguousarray(xs_b.T),
            "xsrT": np.ascontiguousarray(xs_rows.T),
            "xtT": np.ascontiguousarray(xt_b.T),
            "xtrT": np.ascontiguousarray(xt_b[TB * h:TB * h + TB].T),
            "wpack": wp,
        })
    return maps


def run_kernel(inputs, trace=False, **spmd_kwargs):
    nc = _build_nc()
    maps = _in_maps(**inputs)
    res = bass_utils.run_bass_kernel_spmd(
        nc, maps, core_ids=list(range(N_CORES)), trace=trace, **spmd_kwargs)
    adj = np.empty((B, NT, NT), dtype=np.float32)
    for c in range(N_CORES):
        b, h = c // 2, c % 2
        oa = np.asarray(res.results[c]["out_a"], dtype=np.float32)
        ob = np.asarray(res.results[c]["out_b"], dtype=np.float32)
        for li, g in enumerate(GL[h]):
            adj[b, 128 * g:128 * g + 128, :] = oa[128 * li:128 * li + 128]
        adj[b, N + TB * h:N + TB * h + TB, :] = ob
    return adj, res


def kernel(**inputs):
    adj, _ = run_kernel(inputs, trace=False)
    return adj
